# revision 1
# baseline (speedup 1.0000x reference)
"""GQA attention layer (dense_transformer) on 8 Trainium2 NeuronCores.

Tensor-parallel over heads: each core gets 4 q-heads + 1 kv-head (shard of
wq/wk/wv output dims and wo input dim), hidden_states replicated; partial
o_proj outputs are summed on the host (the all-reduce).

Per-core pipeline (all matmuls bf16 with fp32 PSUM accumulation):
  1. qkv projections from host-pretransposed hsT tiles
  2. fused RMSNorm (norm_w folded into host-precomputed RoPE tables) + RoPE
  3. PE-transpose q/k into [d, t] layout
  4. attention: scoresT = k @ qT per 128-row k-tile; exp(scale*x) on ACT;
     causal mask via 0/1 multiply on diagonal straddlers; PV with an
     appended ones-column on V so the softmax denominator comes out of the
     same matmul; normalize after.
  5. o_proj from PE-transposed attention output, fp32 result to DRAM.
"""

import numpy as np
import ml_dtypes

H, KV, D, HID = 32, 8, 128, 4096
B, S = 2, 2048
T = B * S
NCORES = 8
HL = H // NCORES          # 4 q heads per core
QF = HL * D               # 512
EPS = 1e-6
THETA = 10000.0
SCALE = 1.0 / float(np.sqrt(D))

_NC_CACHE = {}


def _build():
    import concourse.bacc as bacc
    import concourse.mybir as mybir
    import concourse.tile as tile
    from concourse.masks import make_identity

    fp32 = mybir.dt.float32
    bf16 = mybir.dt.bfloat16

    nc = bacc.Bacc("TRN2", target_bir_lowering=False)

    hsT = nc.dram_tensor("hsT", [HID, T], bf16, kind="ExternalInput")
    wq = nc.dram_tensor("wq", [HID, QF], bf16, kind="ExternalInput")
    wkv = nc.dram_tensor("wkv", [HID, 2 * D], bf16, kind="ExternalInput")
    wo = nc.dram_tensor("wo", [QF, HID], bf16, kind="ExternalInput")
    cosq = nc.dram_tensor("cosq", [S, D], bf16, kind="ExternalInput")
    sinq = nc.dram_tensor("sinq", [S, D], bf16, kind="ExternalInput")
    cosk = nc.dram_tensor("cosk", [S, D], bf16, kind="ExternalInput")
    sink = nc.dram_tensor("sink", [S, D], bf16, kind="ExternalInput")
    out = nc.dram_tensor("out", [T, HID], fp32, kind="ExternalOutput")

    NT = T // 128            # 32 token tiles
    NTB = S // 128           # 16 token tiles per batch
    NC = HID // 128          # 32 contraction chunks

    with tile.TileContext(nc) as tc:
        with (
            tc.tile_pool(name="persist", bufs=1) as persist,
            tc.tile_pool(name="hst", bufs=2) as hstp,
            tc.tile_pool(name="work", bufs=3) as work,
            tc.tile_pool(name="probs", bufs=4) as probsp,
            tc.tile_pool(name="stats", bufs=8) as stats,
            tc.tile_pool(name="ostage", bufs=3) as ostage,
            tc.tile_pool(name="psA", bufs=2, space="PSUM") as psA,
            tc.tile_pool(name="psB", bufs=4, space="PSUM") as psB,
            tc.tile_pool(name="psT", bufs=2, space="PSUM") as psT,
        ):
            # ---- persistent constants / weights ----
            ident = persist.tile([128, 128], bf16)
            make_identity(nc, ident)
            eps_t = persist.tile([128, 1], fp32)
            nc.vector.memset(eps_t, EPS)

            masks = persist.tile([128, 4, 512], bf16)
            for i in range(4):
                nc.gpsimd.memset(masks[:, i, :], 1.0)
                nc.gpsimd.affine_select(
                    out=masks[:, i, :], in_=masks[:, i, :],
                    compare_op=mybir.AluOpType.is_ge,
                    fill=0.0, base=-128 * i,
                    pattern=[[1, 512]], channel_multiplier=-1,
                )

            wq_sb = persist.tile([128, NC, QF], bf16)
            nc.sync.dma_start(out=wq_sb, in_=wq.rearrange("(c p) f -> p c f", p=128))
            wkv_sb = persist.tile([128, NC, 2 * D], bf16)
            nc.sync.dma_start(out=wkv_sb, in_=wkv.rearrange("(c p) f -> p c f", p=128))
            wo_sb = persist.tile([128, HL, HID], bf16)
            nc.sync.dma_start(out=wo_sb, in_=wo.rearrange("(h p) f -> p h f", p=128))

            tabs = {}
            for name, t in (("cosq", cosq), ("sinq", sinq), ("cosk", cosk), ("sink", sink)):
                tt = persist.tile([128, NTB, D], bf16, name=f"tab_{name}")
                nc.sync.dma_start(out=tt, in_=t.rearrange("(n p) d -> p n d", p=128))
                tabs[name] = tt

            # ---- persistent activations ----
            QT = [persist.tile([128, T], bf16, name=f"QT{h}") for h in range(HL)]
            KT = persist.tile([128, T], bf16)                       # [d, t]
            VA = persist.tile([128, NT, D + 1], bf16)               # [sk, d | 1] per tile
            OT = [persist.tile([128, S], bf16, name=f"OT{h}") for h in range(HL)]

            # ================= phase 1: projections + norm + rope =================
            def norm_rope_transpose(psum_slice, cos_t, sin_t, dstT, tcol):
                ssq = stats.tile([128, 1], fp32, tag="ssq")
                scratch = work.tile([128, 128], bf16, tag="sq")
                nc.scalar.activation(
                    out=scratch, in_=psum_slice,
                    func=mybir.ActivationFunctionType.Square,
                    accum_out=ssq,
                )
                rstd = stats.tile([128, 1], fp32, tag="rstd")
                nc.scalar.activation(
                    out=rstd, in_=ssq, func=mybir.ActivationFunctionType.Sqrt,
                    bias=eps_t, scale=1.0 / D,
                )
                nc.vector.reciprocal(out=rstd, in_=rstd)

                ynorm = work.tile([128, 128], bf16, tag="ynorm")
                shifted = work.tile([128, 128], bf16, tag="shifted")
                nc.vector.tensor_scalar_mul(out=ynorm, in0=psum_slice, scalar1=rstd)
                nc.vector.tensor_scalar_mul(
                    out=shifted[:, 0:64], in0=psum_slice[:, 64:128], scalar1=rstd)
                nc.vector.tensor_scalar_mul(
                    out=shifted[:, 64:128], in0=psum_slice[:, 0:64], scalar1=rstd)
                rot = work.tile([128, 128], bf16, tag="rot")
                nc.vector.tensor_mul(out=rot, in0=ynorm, in1=cos_t)
                nc.vector.tensor_mul(out=shifted, in0=shifted, in1=sin_t)
                nc.vector.tensor_add(out=rot, in0=rot, in1=shifted)

                ptr = psT.tile([128, 128], bf16, tag="tr")
                nc.tensor.transpose(ptr, rot, ident)
                nc.any.tensor_copy(out=dstT[:, tcol:tcol + 128], in_=ptr)

            for i in range(NT):
                si = i % NTB  # position tile within batch
                hst_i = hstp.tile([128, NC, 128], bf16)
                nc.sync.dma_start(
                    out=hst_i,
                    in_=hsT[:, i * 128:(i + 1) * 128].rearrange("(c p) t -> p c t", p=128),
                )
                pq = psA.tile([128, QF], fp32, tag="A")
                pkv = psB.tile([128, 2 * D], fp32, tag="B")
                for c in range(NC):
                    # back-to-back matmuls sharing the same stationary hst
                    # tile so the PE reuses one LDWEIGHTS for both
                    nc.tensor.matmul(pq, hst_i[:, c, :], wq_sb[:, c, :],
                                     start=(c == 0), stop=(c == NC - 1))
                    nc.tensor.matmul(pkv, hst_i[:, c, :], wkv_sb[:, c, :],
                                     start=(c == 0), stop=(c == NC - 1))

                for h in range(HL):
                    norm_rope_transpose(
                        pq[:, h * D:(h + 1) * D],
                        tabs["cosq"][:, si, :], tabs["sinq"][:, si, :],
                        QT[h], i * 128)
                norm_rope_transpose(
                    pkv[:, 0:D],
                    tabs["cosk"][:, si, :], tabs["sink"][:, si, :],
                    KT, i * 128)
                nc.any.tensor_copy(out=VA[:, i, 0:D], in_=pkv[:, D:2 * D])
                nc.vector.memset(VA[:, i, D:D + 1], 1.0)

            # ============ phase 2 + 3: attention and o_proj, per batch ============
            for b in range(B):
                t0 = b * S
                k0 = b * NTB
                for h in range(HL):
                    for j in range(4):  # 512-wide sq tiles
                        qcol = t0 + j * 512
                        n_sk = 4 * (j + 1)
                        opv = [psB.tile([128, D + 1], fp32, tag="B", name=f"opv{s}")
                               for s in range(4)]
                        for k in range(n_sk):
                            ps_s = psA.tile([128, 512], fp32, tag="A")
                            nc.tensor.matmul(
                                ps_s,
                                KT[:, t0 + k * 128: t0 + (k + 1) * 128],
                                QT[h][:, qcol:qcol + 512],
                                start=True, stop=True)
                            pr = probsp.tile([128, 512], bf16, tag="pr")
                            nc.scalar.activation(
                                out=pr, in_=ps_s,
                                func=mybir.ActivationFunctionType.Exp,
                                scale=SCALE)
                            if k >= 4 * j:
                                nc.vector.tensor_mul(
                                    out=pr, in0=pr, in1=masks[:, k - 4 * j, :])
                            for s in range(4):
                                last_k = 4 * j + s
                                if k > last_k:
                                    continue
                                nc.tensor.matmul(
                                    opv[s],
                                    pr[:, s * 128:(s + 1) * 128],
                                    VA[:, k0 + k, :],
                                    start=(k == 0), stop=(k == last_k))
                        for s in range(4):
                            recip = stats.tile([128, 1], fp32, tag="recip")
                            nc.vector.reciprocal(out=recip, in_=opv[s][:, D:D + 1])
                            onorm = work.tile([128, 128], bf16, tag="onorm")
                            nc.vector.tensor_scalar_mul(
                                out=onorm, in0=opv[s][:, 0:D], scalar1=recip)
                            ptr = psT.tile([128, 128], bf16, tag="tr")
                            nc.tensor.transpose(ptr, onorm, ident)
                            tloc = j * 512 + s * 128
                            nc.any.tensor_copy(
                                out=OT[h][:, tloc:tloc + 128], in_=ptr)

                # o_proj for this batch's 16 token tiles
                for it in range(NTB):
                    for n in range(HID // 512):
                        po = psA.tile([128, 512], fp32, tag="A")
                        for h in range(HL):
                            nc.tensor.matmul(
                                po,
                                OT[h][:, it * 128:(it + 1) * 128],
                                wo_sb[:, h, n * 512:(n + 1) * 512],
                                start=(h == 0), stop=(h == HL - 1))
                        ost = ostage.tile([128, 512], fp32, tag="ost")
                        nc.any.tensor_copy(out=ost, in_=po)
                        nc.sync.dma_start(
                            out=out[t0 + it * 128: t0 + (it + 1) * 128,
                                    n * 512:(n + 1) * 512],
                            in_=ost)

    nc.finalize()
    return nc


def _get_nc():
    if "nc" not in _NC_CACHE:
        _NC_CACHE["nc"] = _build()
    return _NC_CACHE["nc"]


def _host_prep(hidden_states, wq, wk, wv, wo, q_norm_w, k_norm_w, position_ids):
    bf = ml_dtypes.bfloat16
    hs = np.asarray(hidden_states, dtype=np.float32).reshape(T, HID)
    hsT = np.ascontiguousarray(hs.T).astype(bf)

    # RoPE tables with norm weights folded in (positions are identical
    # across batches for this problem's arange position_ids).
    pos = np.asarray(position_ids)[0].astype(np.float64)
    inv_freq = 1.0 / (THETA ** (np.arange(0, D, 2, dtype=np.float64) / D))
    ang = pos[:, None] * inv_freq
    emb = np.concatenate([ang, ang], axis=-1)
    cos = np.cos(emb).astype(np.float32)
    sin = np.sin(emb).astype(np.float32)

    def fold(w):
        w = np.asarray(w, dtype=np.float32)
        w_shift = np.concatenate([w[D // 2:], w[:D // 2]])
        sgn = np.concatenate([-np.ones(D // 2, np.float32), np.ones(D // 2, np.float32)])
        return (cos * w).astype(bf), (sin * w_shift * sgn).astype(bf)

    cq, sq_ = fold(q_norm_w)
    ck, sk_ = fold(k_norm_w)

    wq = np.asarray(wq, dtype=np.float32)
    wk = np.asarray(wk, dtype=np.float32)
    wv = np.asarray(wv, dtype=np.float32)
    wo = np.asarray(wo, dtype=np.float32)

    in_maps = []
    for c in range(NCORES):
        qs = slice(c * QF, (c + 1) * QF)
        ks = slice(c * D, (c + 1) * D)
        in_maps.append({
            "hsT": hsT,
            "wq": np.ascontiguousarray(wq[:, qs]).astype(bf),
            "wkv": np.ascontiguousarray(
                np.concatenate([wk[:, ks], wv[:, ks]], axis=1)).astype(bf),
            "wo": np.ascontiguousarray(wo[qs, :]).astype(bf),
            "cosq": cq, "sinq": sq_, "cosk": ck, "sink": sk_,
        })
    return in_maps


def kernel(hidden_states, wq, wk, wv, wo, q_norm_w, k_norm_w, position_ids,
           _trace=False):
    from concourse.bass_utils import run_bass_kernel_spmd

    nc = _get_nc()
    in_maps = _host_prep(hidden_states, wq, wk, wv, wo,
                         q_norm_w, k_norm_w, position_ids)
    res = run_bass_kernel_spmd(nc, in_maps, core_ids=list(range(NCORES)),
                               trace=_trace)
    total = np.zeros((T, HID), dtype=np.float32)
    for r in res.results:
        total += r["out"]
    out = total.reshape(B, S, HID)
    if _trace:
        return out, res
    return out



# revision 11
# speedup vs baseline: 1.0863x; 1.0863x over previous
"""GQA attention layer (dense_transformer) on 8 Trainium2 NeuronCores.

Tensor-parallel over heads: each core gets 4 q-heads + 1 kv-head (shard of
wq/wk/wv output dims and wo input dim), hidden_states replicated; partial
o_proj outputs are summed on the host (the all-reduce).

Per-core pipeline (all matmuls bf16 with fp32 PSUM accumulation):
  phase 1: qkv projections from host-pretransposed hsT tiles; fused RMSNorm
    (norm_w folded into host-precomputed RoPE tables) + RoPE; PE-transpose
    q/k into [d, t]. Transposes for tile i are emitted after tile i+1's
    matmuls so the PE never waits on the vector-engine norm chain.
  phase 2 (per batch, per 512-wide sq block j, per head):
    scoresT[sk,sq] = k_tile @ qT per 128-row k-tile; exp on ACT; causal
    mask via 0/1 multiply on diagonal straddlers; PV with STATIONARY V:
    opvT[d,sq] += V_k^T @ probs_k -- one 512-wide matmul per k-tile, and
    the output lands pre-transposed for o_proj (no output transpose).
    Softmax denominator: probs accumulated on DVE (fp32), one ones-vector
    matmul -> denom row, reciprocal (DVE), partition_broadcast (Pool),
    normalize fused into the opvT PSUM->SBUF copy.
    o_proj matmul groups for block j-1 are interleaved into the k-loops
    as always-ready PE filler so the tensor engine never idles on exp.
"""

import numpy as np
import ml_dtypes

H, KV, D, HID = 32, 8, 128, 4096
B, S = 2, 2048
T = B * S
NCORES = 8
HL = H // NCORES          # 4 q heads per core
QF = HL * D               # 512
EPS = 1e-6
THETA = 10000.0
SCALE = 1.0 / float(np.sqrt(D))

_NC_CACHE = {}


def _build():
    import concourse.bacc as bacc
    import concourse.mybir as mybir
    import concourse.tile as tile
    from concourse.masks import make_identity

    fp32 = mybir.dt.float32
    bf16 = mybir.dt.bfloat16

    nc = bacc.Bacc("TRN2", target_bir_lowering=False)

    hsT = nc.dram_tensor("hsT", [HID, T], bf16, kind="ExternalInput")
    wq = nc.dram_tensor("wq", [HID, QF], bf16, kind="ExternalInput")
    wkv = nc.dram_tensor("wkv", [HID, 2 * D], bf16, kind="ExternalInput")
    wo = nc.dram_tensor("wo", [QF, HID], bf16, kind="ExternalInput")
    cosq = nc.dram_tensor("cosq", [S, D], bf16, kind="ExternalInput")
    sinq = nc.dram_tensor("sinq", [S, D], bf16, kind="ExternalInput")
    cosk = nc.dram_tensor("cosk", [S, D], bf16, kind="ExternalInput")
    sink = nc.dram_tensor("sink", [S, D], bf16, kind="ExternalInput")
    out = nc.dram_tensor("out", [T, HID], fp32, kind="ExternalOutput")

    NT = T // 128            # 32 token tiles
    NTB = S // 128           # 16 token tiles per batch
    NC = HID // 128          # 32 contraction chunks
    NJ = 4                   # 512-wide sq blocks per batch
    NO = HID // 512          # o_proj output chunks

    with tile.TileContext(nc) as tc:
        with (
            tc.tile_pool(name="persist", bufs=1) as persist,
            tc.tile_pool(name="hst", bufs=2) as hstp,
            tc.tile_pool(name="work", bufs=3) as work,
            tc.tile_pool(name="probs", bufs=6) as probsp,
            tc.tile_pool(name="stats", bufs=8) as stats,
            tc.tile_pool(name="otb", bufs=3) as otbp,
            tc.tile_pool(name="prsp", bufs=2) as prsp,
            tc.tile_pool(name="prsbp", bufs=2) as prsbp,
            tc.tile_pool(name="rowp", bufs=2) as rowp,
            tc.tile_pool(name="rbcp", bufs=2) as rbcp,
            tc.tile_pool(name="ostage", bufs=3) as ostage,
            tc.tile_pool(name="psS", bufs=2, space="PSUM") as psS,
            tc.tile_pool(name="psV", bufs=2, space="PSUM") as psV,
            tc.tile_pool(name="psO", bufs=2, space="PSUM") as psO,
            tc.tile_pool(name="psT", bufs=2, space="PSUM") as psT,
        ):
            # ---- persistent constants / weights ----
            ident = persist.tile([128, 128], bf16)
            make_identity(nc, ident)
            eps_t = persist.tile([128, 1], fp32)
            nc.vector.memset(eps_t, EPS)
            ones_col = persist.tile([128, 1], bf16)
            nc.vector.memset(ones_col, 1.0)

            masks = persist.tile([128, 4, 512], bf16)
            for i in range(4):
                nc.gpsimd.memset(masks[:, i, :], 1.0)
                nc.gpsimd.affine_select(
                    out=masks[:, i, :], in_=masks[:, i, :],
                    compare_op=mybir.AluOpType.is_ge,
                    fill=0.0, base=-128 * i,
                    pattern=[[1, 512]], channel_multiplier=-1,
                )

            # chunked weight loads so the first matmuls start early
            wq_sb = persist.tile([128, NC, QF], bf16)
            wkv_sb = persist.tile([128, NC, 2 * D], bf16)
            wq_r = wq.rearrange("(c p) f -> p c f", p=128)
            wkv_r = wkv.rearrange("(c p) f -> p c f", p=128)
            for c0 in range(0, NC, 8):
                nc.sync.dma_start(out=wq_sb[:, c0:c0 + 8, :],
                                  in_=wq_r[:, c0:c0 + 8, :])
                nc.sync.dma_start(out=wkv_sb[:, c0:c0 + 8, :],
                                  in_=wkv_r[:, c0:c0 + 8, :])
            wo_sb = persist.tile([128, HL, HID], bf16)
            nc.sync.dma_start(out=wo_sb, in_=wo.rearrange("(h p) f -> p h f", p=128))

            tabs = {}
            for name, t in (("cosq", cosq), ("sinq", sinq), ("cosk", cosk), ("sink", sink)):
                tt = persist.tile([128, NTB, D], bf16, name=f"tab_{name}")
                nc.sync.dma_start(out=tt, in_=t.rearrange("(n p) d -> p n d", p=128))
                tabs[name] = tt

            # ---- persistent activations ----
            QT = [persist.tile([128, T], bf16, name=f"QT{h}") for h in range(HL)]
            KT = persist.tile([128, T], bf16)                       # [d, t]
            VA = persist.tile([128, NT, D], bf16)                   # [sk, d] per tile

            # ================= phase 1: projections + norm + rope =================
            def norm_rope_transpose(psum_slice, cos_t, sin_t, dstT, tcol):
                ssq = stats.tile([128, 1], fp32, tag="ssq")
                scratch = work.tile([128, 128], bf16, tag="sq")
                nc.scalar.activation(
                    out=scratch, in_=psum_slice,
                    func=mybir.ActivationFunctionType.Square,
                    accum_out=ssq,
                )
                rstd = stats.tile([128, 1], fp32, tag="rstd")
                nc.scalar.activation(
                    out=rstd, in_=ssq, func=mybir.ActivationFunctionType.Sqrt,
                    bias=eps_t, scale=1.0 / D,
                )
                nc.vector.reciprocal(out=rstd, in_=rstd)

                ynorm = work.tile([128, 128], bf16, tag="ynorm")
                shifted = work.tile([128, 128], bf16, tag="shifted")
                nc.vector.tensor_scalar_mul(out=ynorm, in0=psum_slice, scalar1=rstd)
                nc.vector.tensor_scalar_mul(
                    out=shifted[:, 0:64], in0=psum_slice[:, 64:128], scalar1=rstd)
                nc.vector.tensor_scalar_mul(
                    out=shifted[:, 64:128], in0=psum_slice[:, 0:64], scalar1=rstd)
                rot = work.tile([128, 128], bf16, tag="rot")
                nc.vector.tensor_mul(out=rot, in0=ynorm, in1=cos_t)
                nc.vector.tensor_mul(out=shifted, in0=shifted, in1=sin_t)
                nc.vector.tensor_add(out=rot, in0=rot, in1=shifted)

                ptr = psT.tile([128, 128], bf16, tag="tr")
                nc.tensor.transpose(ptr, rot, ident)
                nc.any.tensor_copy(out=dstT[:, tcol:tcol + 128], in_=ptr)

            def finish_tile(pq, pkv, i):
                si = i % NTB
                for h in range(HL):
                    norm_rope_transpose(
                        pq[:, h * D:(h + 1) * D],
                        tabs["cosq"][:, si, :], tabs["sinq"][:, si, :],
                        QT[h], i * 128)
                norm_rope_transpose(
                    pkv[:, 0:D],
                    tabs["cosk"][:, si, :], tabs["sink"][:, si, :],
                    KT, i * 128)
                nc.any.tensor_copy(out=VA[:, i, :], in_=pkv[:, D:2 * D])

            pend = None
            for i in range(NT):
                hst_i = hstp.tile([128, NC, 128], bf16)
                nc.sync.dma_start(
                    out=hst_i,
                    in_=hsT[:, i * 128:(i + 1) * 128].rearrange("(c p) t -> p c t", p=128),
                )
                pq = psS.tile([128, QF], fp32, tag="S")
                pkv = psV.tile([128, 2 * D], fp32, tag="V")
                for c in range(NC):
                    nc.tensor.matmul(pq, hst_i[:, c, :], wq_sb[:, c, :],
                                     start=(c == 0), stop=(c == NC - 1))
                    nc.tensor.matmul(pkv, hst_i[:, c, :], wkv_sb[:, c, :],
                                     start=(c == 0), stop=(c == NC - 1))
                if pend is not None:
                    finish_tile(*pend)
                pend = (pq, pkv, i)
            finish_tile(*pend)

            # ============ phase 2: attention with interleaved o_proj ============
            fill = []          # pending o_proj emitters (always-ready PE work)

            def drain_fill(n):
                for _ in range(min(n, len(fill))):
                    fill.pop(0)()

            def make_oproj_group(ot_blk, b, j, it, n):
                def emit():
                    po = psO.tile([128, 512], fp32, tag="O")
                    for h in range(HL):
                        nc.tensor.matmul(
                            po,
                            ot_blk[:, h, it * 128:(it + 1) * 128],
                            wo_sb[:, h, n * 512:(n + 1) * 512],
                            start=(h == 0), stop=(h == HL - 1))
                    ost = ostage.tile([128, 512], fp32, tag="ost")
                    nc.scalar.copy(out=ost, in_=po)
                    t0 = b * S + j * 512 + it * 128
                    nc.sync.dma_start(
                        out=out[t0:t0 + 128, n * 512:(n + 1) * 512], in_=ost)
                return emit

            for b in range(B):
                t0 = b * S
                k0 = b * NTB
                for j in range(NJ):
                    qcol = t0 + j * 512
                    K = 4 * (j + 1)
                    ot_blk = otbp.tile([128, HL, 512], bf16, name=f"otb")
                    for h in range(HL):
                        prs = prsp.tile([128, 512], fp32, tag="prsum")
                        opvT = psV.tile([128, 512], fp32, tag="V")
                        prq = []   # pending (k, pr) for lagged PV
                        for k in range(K):
                            ps_s = psS.tile([128, 512], fp32, tag="S")
                            nc.tensor.matmul(
                                ps_s,
                                KT[:, t0 + k * 128: t0 + (k + 1) * 128],
                                QT[h][:, qcol:qcol + 512],
                                start=True, stop=True)
                            pr = probsp.tile([128, 512], bf16, tag="pr")
                            nc.scalar.activation(
                                out=pr, in_=ps_s,
                                func=mybir.ActivationFunctionType.Exp,
                                scale=SCALE)
                            if k >= 4 * j:
                                nc.vector.tensor_mul(
                                    out=pr, in0=pr, in1=masks[:, k - 4 * j, :])
                            if k == 0:
                                nc.vector.tensor_copy(out=prs, in_=pr)
                            else:
                                nc.vector.tensor_add(out=prs, in0=prs, in1=pr)
                            prq.append((k, pr))
                            drain_fill(1)
                            if len(prq) > 2:
                                kk, prk = prq.pop(0)
                                nc.tensor.matmul(
                                    opvT, VA[:, k0 + kk, :], prk,
                                    start=(kk == 0), stop=(kk == K - 1))
                        for kk, prk in prq:
                            nc.tensor.matmul(
                                opvT, VA[:, k0 + kk, :], prk,
                                start=(kk == 0), stop=(kk == K - 1))
                        # denominator: ones^T @ prsum -> [1, 512], recip,
                        # broadcast across partitions, normalize during the
                        # PSUM->SBUF copy of opvT.
                        prsb = prsbp.tile([128, 512], bf16, tag="prsb")
                        nc.vector.tensor_copy(out=prsb, in_=prs)
                        dn = psT.tile([1, 512], fp32, tag="tr")
                        nc.tensor.matmul(dn, ones_col, prsb,
                                         start=True, stop=True)
                        rrow = rowp.tile([1, 512], fp32, tag="rrow")
                        nc.vector.reciprocal(out=rrow, in_=dn)
                        rbc = rbcp.tile([128, 512], fp32, tag="rbc")
                        nc.gpsimd.partition_broadcast(rbc, rrow)
                        nc.vector.tensor_mul(
                            out=ot_blk[:, h, :], in0=opvT, in1=rbc)
                    fill.extend(
                        make_oproj_group(ot_blk, b, j, it, n)
                        for it in range(4) for n in range(NO))
            drain_fill(len(fill))

    nc.finalize()
    return nc


def _get_nc():
    if "nc" not in _NC_CACHE:
        _NC_CACHE["nc"] = _build()
    return _NC_CACHE["nc"]


def _host_prep(hidden_states, wq, wk, wv, wo, q_norm_w, k_norm_w, position_ids):
    bf = ml_dtypes.bfloat16
    hs = np.asarray(hidden_states, dtype=np.float32).reshape(T, HID)
    hsT = np.ascontiguousarray(hs.T).astype(bf)

    # RoPE tables with norm weights folded in (positions are identical
    # across batches for this problem's arange position_ids).
    pos = np.asarray(position_ids)[0].astype(np.float64)
    inv_freq = 1.0 / (THETA ** (np.arange(0, D, 2, dtype=np.float64) / D))
    ang = pos[:, None] * inv_freq
    emb = np.concatenate([ang, ang], axis=-1)
    cos = np.cos(emb).astype(np.float32)
    sin = np.sin(emb).astype(np.float32)

    def fold(w):
        w = np.asarray(w, dtype=np.float32)
        w_shift = np.concatenate([w[D // 2:], w[:D // 2]])
        sgn = np.concatenate([-np.ones(D // 2, np.float32), np.ones(D // 2, np.float32)])
        return (cos * w).astype(bf), (sin * w_shift * sgn).astype(bf)

    cq, sq_ = fold(q_norm_w)
    ck, sk_ = fold(k_norm_w)

    wq = np.asarray(wq, dtype=np.float32)
    wk = np.asarray(wk, dtype=np.float32)
    wv = np.asarray(wv, dtype=np.float32)
    wo = np.asarray(wo, dtype=np.float32)

    in_maps = []
    for c in range(NCORES):
        qs = slice(c * QF, (c + 1) * QF)
        ks = slice(c * D, (c + 1) * D)
        in_maps.append({
            "hsT": hsT,
            "wq": np.ascontiguousarray(wq[:, qs]).astype(bf),
            "wkv": np.ascontiguousarray(
                np.concatenate([wk[:, ks], wv[:, ks]], axis=1)).astype(bf),
            "wo": np.ascontiguousarray(wo[qs, :]).astype(bf),
            "cosq": cq, "sinq": sq_, "cosk": ck, "sink": sk_,
        })
    return in_maps


def kernel(hidden_states, wq, wk, wv, wo, q_norm_w, k_norm_w, position_ids,
           _trace=False):
    from concourse.bass_utils import run_bass_kernel_spmd

    nc = _get_nc()
    in_maps = _host_prep(hidden_states, wq, wk, wv, wo,
                         q_norm_w, k_norm_w, position_ids)
    res = run_bass_kernel_spmd(nc, in_maps, core_ids=list(range(NCORES)),
                               trace=_trace)
    total = np.zeros((T, HID), dtype=np.float32)
    for r in res.results:
        total += r["out"]
    out = total.reshape(B, S, HID)
    if _trace:
        return out, res
    return out


# revision 13
# speedup vs baseline: 1.2148x; 1.1183x over previous
"""GQA attention layer (dense_transformer) on 8 Trainium2 NeuronCores.

Tensor-parallel over heads: each core gets 4 q-heads + 1 kv-head (shard of
wq/wk/wv output dims and wo input dim), hidden_states replicated; partial
o_proj outputs are summed on the host (the all-reduce).

Per-core pipeline (all matmuls bf16 with fp32 PSUM accumulation):
  phase 1: qkv projections from host-pretransposed hsT tiles; fused RMSNorm
    (norm_w folded into host-precomputed RoPE tables) + RoPE; PE-transpose
    q/k into [d, t]. Transposes for tile i are emitted after tile i+1's
    matmuls so the PE never waits on the vector-engine norm chain.
  phase 2 (per batch, per 512-wide sq block j, per head):
    scoresT[sk,sq] = k_tile @ qT, two k-tiles paired into one 2-bank PSUM
    tile so a single ACT exp covers both (halves ACT instruction count);
    causal mask via 0/1 multiply on diagonal straddlers; PV with STATIONARY
    V: opvT[d,sq] += V_k^T @ probs_k -- one 512-wide matmul per k-tile whose
    output lands pre-transposed for o_proj (no output transposes).
    Softmax denominator: probs pair-summed on DVE (bf16), folded, one
    ones-vector matmul -> denom row, reciprocal_approx_fast (DVE),
    partition_broadcast (Pool), normalize fused into the opvT PSUM->SBUF
    copy. o_proj matmul groups for block j-1 are interleaved into the
    k-loops as always-ready PE filler so the tensor engine never idles on
    exp; their PSUM->SBUF staging copies alternate ACT/Pool.
"""

import numpy as np
import ml_dtypes

H, KV, D, HID = 32, 8, 128, 4096
B, S = 2, 2048
T = B * S
NCORES = 8
HL = H // NCORES          # 4 q heads per core
QF = HL * D               # 512
EPS = 1e-6
THETA = 10000.0
SCALE = 1.0 / float(np.sqrt(D))

_NC_CACHE = {}


def _build():
    import concourse.bacc as bacc
    import concourse.mybir as mybir
    import concourse.tile as tile
    from concourse.masks import make_identity

    fp32 = mybir.dt.float32
    bf16 = mybir.dt.bfloat16

    nc = bacc.Bacc("TRN2", target_bir_lowering=False)

    hsT = nc.dram_tensor("hsT", [HID, T], bf16, kind="ExternalInput")
    wq = nc.dram_tensor("wq", [HID, QF], bf16, kind="ExternalInput")
    wkv = nc.dram_tensor("wkv", [HID, 2 * D], bf16, kind="ExternalInput")
    wo = nc.dram_tensor("wo", [QF, HID], bf16, kind="ExternalInput")
    cosq = nc.dram_tensor("cosq", [S, D], bf16, kind="ExternalInput")
    sinq = nc.dram_tensor("sinq", [S, D], bf16, kind="ExternalInput")
    cosk = nc.dram_tensor("cosk", [S, D], bf16, kind="ExternalInput")
    sink = nc.dram_tensor("sink", [S, D], bf16, kind="ExternalInput")
    out = nc.dram_tensor("out", [T, HID], fp32, kind="ExternalOutput")

    NT = T // 128            # 32 token tiles
    NTB = S // 128           # 16 token tiles per batch
    NC = HID // 128          # 32 contraction chunks
    NJ = 4                   # 512-wide sq blocks per batch
    NO = HID // 512          # o_proj output chunks

    with tile.TileContext(nc) as tc:
        with (
            tc.tile_pool(name="persist", bufs=1) as persist,
            tc.tile_pool(name="hst", bufs=2) as hstp,
            tc.tile_pool(name="work", bufs=3) as work,
            tc.tile_pool(name="probs", bufs=3) as probsp,
            tc.tile_pool(name="stats", bufs=8) as stats,
            tc.tile_pool(name="otb", bufs=3) as otbp,
            tc.tile_pool(name="prsp", bufs=2) as prsp,
            tc.tile_pool(name="prsbp", bufs=2) as prsbp,
            tc.tile_pool(name="rowp", bufs=2) as rowp,
            tc.tile_pool(name="rbcp", bufs=2) as rbcp,
            tc.tile_pool(name="ostage", bufs=4) as ostage,
        ):
            # ---- persistent constants / weights ----
            ident = persist.tile([128, 128], bf16)
            make_identity(nc, ident)
            eps_t = persist.tile([128, 1], fp32)
            nc.vector.memset(eps_t, EPS)
            ones_col = persist.tile([128, 1], bf16)
            nc.vector.memset(ones_col, 1.0)

            masks = persist.tile([128, 4, 512], bf16)
            for i in range(4):
                nc.gpsimd.memset(masks[:, i, :], 1.0)
                nc.gpsimd.affine_select(
                    out=masks[:, i, :], in_=masks[:, i, :],
                    compare_op=mybir.AluOpType.is_ge,
                    fill=0.0, base=-128 * i,
                    pattern=[[1, 512]], channel_multiplier=-1,
                )

            # chunked weight loads so the first matmuls start early
            wq_sb = persist.tile([128, NC, QF], bf16)
            wkv_sb = persist.tile([128, NC, 2 * D], bf16)
            wq_r = wq.rearrange("(c p) f -> p c f", p=128)
            wkv_r = wkv.rearrange("(c p) f -> p c f", p=128)
            for c0 in range(0, NC, 8):
                nc.sync.dma_start(out=wq_sb[:, c0:c0 + 8, :],
                                  in_=wq_r[:, c0:c0 + 8, :])
                nc.sync.dma_start(out=wkv_sb[:, c0:c0 + 8, :],
                                  in_=wkv_r[:, c0:c0 + 8, :])
            wo_sb = persist.tile([128, HL, HID], bf16)
            nc.sync.dma_start(out=wo_sb, in_=wo.rearrange("(h p) f -> p h f", p=128))

            tabs = {}
            for name, t in (("cosq", cosq), ("sinq", sinq), ("cosk", cosk), ("sink", sink)):
                tt = persist.tile([128, NTB, D], bf16, name=f"tab_{name}")
                nc.sync.dma_start(out=tt, in_=t.rearrange("(n p) d -> p n d", p=128))
                tabs[name] = tt

            # ---- persistent activations ----
            QT = [persist.tile([128, T], bf16, name=f"QT{h}") for h in range(HL)]
            KT = persist.tile([128, T], bf16)                       # [d, t]
            VA = persist.tile([128, NT, D], bf16)                   # [sk, d] per tile

            # ================= phase 1: projections + norm + rope =================
            with (
                tc.tile_pool(name="psQ", bufs=3, space="PSUM") as psQ,
                tc.tile_pool(name="psKV", bufs=3, space="PSUM") as psKV,
                tc.tile_pool(name="psT", bufs=2, space="PSUM") as psT,
            ):
                def norm_rope_transpose(psum_slice, cos_t, sin_t, dstT, tcol):
                    ssq = stats.tile([128, 1], fp32, tag="ssq")
                    scratch = work.tile([128, 128], bf16, tag="sq")
                    nc.scalar.activation(
                        out=scratch, in_=psum_slice,
                        func=mybir.ActivationFunctionType.Square,
                        accum_out=ssq,
                    )
                    rstd = stats.tile([128, 1], fp32, tag="rstd")
                    nc.scalar.activation(
                        out=rstd, in_=ssq, func=mybir.ActivationFunctionType.Sqrt,
                        bias=eps_t, scale=1.0 / D,
                    )
                    nc.vector.reciprocal(out=rstd, in_=rstd)

                    ynorm = work.tile([128, 128], bf16, tag="ynorm")
                    shifted = work.tile([128, 128], bf16, tag="shifted")
                    nc.vector.tensor_scalar_mul(out=ynorm, in0=psum_slice, scalar1=rstd)
                    nc.vector.tensor_scalar_mul(
                        out=shifted[:, 0:64], in0=psum_slice[:, 64:128], scalar1=rstd)
                    nc.vector.tensor_scalar_mul(
                        out=shifted[:, 64:128], in0=psum_slice[:, 0:64], scalar1=rstd)
                    rot = work.tile([128, 128], bf16, tag="rot")
                    nc.vector.tensor_mul(out=rot, in0=ynorm, in1=cos_t)
                    nc.vector.tensor_mul(out=shifted, in0=shifted, in1=sin_t)
                    nc.vector.tensor_add(out=rot, in0=rot, in1=shifted)

                    ptr = psT.tile([128, 128], bf16, tag="tr")
                    nc.tensor.transpose(ptr, rot, ident)
                    nc.any.tensor_copy(out=dstT[:, tcol:tcol + 128], in_=ptr)

                def finish_tile(pq, pkv, i):
                    si = i % NTB
                    for h in range(HL):
                        norm_rope_transpose(
                            pq[:, h * D:(h + 1) * D],
                            tabs["cosq"][:, si, :], tabs["sinq"][:, si, :],
                            QT[h], i * 128)
                    norm_rope_transpose(
                        pkv[:, 0:D],
                        tabs["cosk"][:, si, :], tabs["sink"][:, si, :],
                        KT, i * 128)
                    nc.any.tensor_copy(out=VA[:, i, :], in_=pkv[:, D:2 * D])

                pend = None
                for i in range(NT):
                    hst_i = hstp.tile([128, NC, 128], bf16)
                    nc.sync.dma_start(
                        out=hst_i,
                        in_=hsT[:, i * 128:(i + 1) * 128].rearrange(
                            "(c p) t -> p c t", p=128),
                    )
                    pq = psQ.tile([128, QF], fp32, tag="Q")
                    pkv = psKV.tile([128, 2 * D], fp32, tag="KV")
                    for c in range(NC):
                        nc.tensor.matmul(pq, hst_i[:, c, :], wq_sb[:, c, :],
                                         start=(c == 0), stop=(c == NC - 1))
                        nc.tensor.matmul(pkv, hst_i[:, c, :], wkv_sb[:, c, :],
                                         start=(c == 0), stop=(c == NC - 1))
                    if pend is not None:
                        finish_tile(*pend)
                    pend = (pq, pkv, i)
                finish_tile(*pend)

            # ============ phase 2: attention with interleaved o_proj ============
            with (
                tc.tile_pool(name="psS", bufs=2, space="PSUM") as psS,
                tc.tile_pool(name="psV", bufs=2, space="PSUM") as psV,
                tc.tile_pool(name="psO", bufs=2, space="PSUM") as psO,
            ):
                fill = []      # pending o_proj emitters (always-ready PE work)
                copy_flip = [0]

                def drain_fill(n):
                    for _ in range(min(n, len(fill))):
                        fill.pop(0)()

                def make_oproj_group(ot_blk, b, j, it, n):
                    def emit():
                        po = psO.tile([128, 512], fp32, tag="O")
                        for h in range(HL):
                            nc.tensor.matmul(
                                po,
                                ot_blk[:, h, it * 128:(it + 1) * 128],
                                wo_sb[:, h, n * 512:(n + 1) * 512],
                                start=(h == 0), stop=(h == HL - 1))
                        ost = ostage.tile([128, 512], fp32, tag="ost")
                        copy_flip[0] ^= 1
                        if copy_flip[0]:
                            nc.scalar.copy(out=ost, in_=po)
                        else:
                            nc.vector.tensor_copy(out=ost, in_=po)
                        t0 = b * S + j * 512 + it * 128
                        nc.sync.dma_start(
                            out=out[t0:t0 + 128, n * 512:(n + 1) * 512], in_=ost)
                    return emit

                for b in range(B):
                    t0 = b * S
                    k0 = b * NTB
                    for j in range(NJ):
                        qcol = t0 + j * 512
                        K = 4 * (j + 1)
                        NP = K // 2
                        ot_blk = otbp.tile([128, HL, 512], bf16, name="otb")
                        for h in range(HL):
                            prs = prsp.tile([128, 2, 512], bf16, tag="prsum")
                            opvT = psV.tile([128, 512], fp32, tag="V")
                            pvq = []   # pending (k, pr_slice) for lagged PV
                            for p in range(NP):
                                spair = psS.tile([128, 2, 512], fp32, tag="S")
                                for half in range(2):
                                    k = 2 * p + half
                                    nc.tensor.matmul(
                                        spair[:, half, :],
                                        KT[:, t0 + k * 128: t0 + (k + 1) * 128],
                                        QT[h][:, qcol:qcol + 512],
                                        start=True, stop=True)
                                prpair = probsp.tile([128, 2, 512], bf16, tag="pr")
                                nc.scalar.activation(
                                    out=prpair, in_=spair,
                                    func=mybir.ActivationFunctionType.Exp,
                                    scale=SCALE)
                                for half in range(2):
                                    k = 2 * p + half
                                    if k >= 4 * j:
                                        nc.vector.tensor_mul(
                                            out=prpair[:, half, :],
                                            in0=prpair[:, half, :],
                                            in1=masks[:, k - 4 * j, :])
                                if p == 0:
                                    nc.vector.tensor_copy(out=prs, in_=prpair)
                                else:
                                    nc.vector.tensor_add(out=prs, in0=prs, in1=prpair)
                                pvq.append((2 * p, prpair[:, 0, :]))
                                pvq.append((2 * p + 1, prpair[:, 1, :]))
                                drain_fill(2)
                                while len(pvq) > 2:
                                    kk, prk = pvq.pop(0)
                                    nc.tensor.matmul(
                                        opvT, VA[:, k0 + kk, :], prk,
                                        start=(kk == 0), stop=(kk == K - 1))
                            for kk, prk in pvq:
                                nc.tensor.matmul(
                                    opvT, VA[:, k0 + kk, :], prk,
                                    start=(kk == 0), stop=(kk == K - 1))
                            # denominator: fold pair halves, ones^T @ prsb ->
                            # [1,512], fast reciprocal, broadcast, normalize
                            # during the opvT PSUM->SBUF copy.
                            prsb = prsbp.tile([128, 512], bf16, tag="prsb")
                            nc.vector.tensor_add(
                                out=prsb, in0=prs[:, 0, :], in1=prs[:, 1, :])
                            dn = psV.tile([1, 512], fp32, tag="V")
                            nc.tensor.matmul(dn, ones_col, prsb,
                                             start=True, stop=True)
                            rrow = rowp.tile([1, 512], fp32, tag="rrow")
                            nc.vector.reciprocal_approx_fast(out=rrow, in_=dn)
                            rbc = rbcp.tile([128, 512], fp32, tag="rbc")
                            nc.gpsimd.partition_broadcast(rbc, rrow)
                            nc.vector.tensor_mul(
                                out=ot_blk[:, h, :], in0=opvT, in1=rbc)
                        fill.extend(
                            make_oproj_group(ot_blk, b, j, it, n)
                            for it in range(4) for n in range(NO))
                drain_fill(len(fill))

    nc.finalize()
    return nc


def _get_nc():
    if "nc" not in _NC_CACHE:
        _NC_CACHE["nc"] = _build()
    return _NC_CACHE["nc"]


def _host_prep(hidden_states, wq, wk, wv, wo, q_norm_w, k_norm_w, position_ids):
    bf = ml_dtypes.bfloat16
    hs = np.asarray(hidden_states, dtype=np.float32).reshape(T, HID)
    hsT = np.ascontiguousarray(hs.T).astype(bf)

    # RoPE tables with norm weights folded in (positions are identical
    # across batches for this problem's arange position_ids).
    pos = np.asarray(position_ids)[0].astype(np.float64)
    inv_freq = 1.0 / (THETA ** (np.arange(0, D, 2, dtype=np.float64) / D))
    ang = pos[:, None] * inv_freq
    emb = np.concatenate([ang, ang], axis=-1)
    cos = np.cos(emb).astype(np.float32)
    sin = np.sin(emb).astype(np.float32)

    def fold(w):
        w = np.asarray(w, dtype=np.float32)
        w_shift = np.concatenate([w[D // 2:], w[:D // 2]])
        sgn = np.concatenate([-np.ones(D // 2, np.float32), np.ones(D // 2, np.float32)])
        return (cos * w).astype(bf), (sin * w_shift * sgn).astype(bf)

    cq, sq_ = fold(q_norm_w)
    ck, sk_ = fold(k_norm_w)

    wq = np.asarray(wq, dtype=np.float32)
    wk = np.asarray(wk, dtype=np.float32)
    wv = np.asarray(wv, dtype=np.float32)
    wo = np.asarray(wo, dtype=np.float32)

    in_maps = []
    for c in range(NCORES):
        qs = slice(c * QF, (c + 1) * QF)
        ks = slice(c * D, (c + 1) * D)
        in_maps.append({
            "hsT": hsT,
            "wq": np.ascontiguousarray(wq[:, qs]).astype(bf),
            "wkv": np.ascontiguousarray(
                np.concatenate([wk[:, ks], wv[:, ks]], axis=1)).astype(bf),
            "wo": np.ascontiguousarray(wo[qs, :]).astype(bf),
            "cosq": cq, "sinq": sq_, "cosk": ck, "sink": sk_,
        })
    return in_maps


def kernel(hidden_states, wq, wk, wv, wo, q_norm_w, k_norm_w, position_ids,
           _trace=False):
    from concourse.bass_utils import run_bass_kernel_spmd

    nc = _get_nc()
    in_maps = _host_prep(hidden_states, wq, wk, wv, wo,
                         q_norm_w, k_norm_w, position_ids)
    res = run_bass_kernel_spmd(nc, in_maps, core_ids=list(range(NCORES)),
                               trace=_trace)
    total = np.zeros((T, HID), dtype=np.float32)
    for r in res.results:
        total += r["out"]
    out = total.reshape(B, S, HID)
    if _trace:
        return out, res
    return out
